# revision 72
# baseline (speedup 1.0000x reference)
"""Trainium2 Bass kernel for softclamped multi-head attention.

Full computation (matches the jax reference):
  x = rmsnorm(tokens) * norm_w
  q = x @ Wq ; k, v = split(x @ Wkv)
  q, k head-l2norm * (gamma+1)*sqrt(d)
  sim = tanh((q k^T)/50)*50 / sqrt(d);  attn = softmax(sim);  out = (attn v) @ Wo

Sharding: 8 cores; core c -> batch c//2, head-group c%2 (8 of 16 heads).
Each core computes a partial output (its head-group's contribution to its
batch); host sums the two partials per batch.

Performance design (per core, engines at: PE 2.4GHz, ACT 1.2GHz, DVE 0.96GHz):
  - The ACT engine is the irreducible bottleneck: tanh+exp over the full
    8-head 2048x2048 score tensor = 2 passes x 262144 elem/partition
    ~ 500us.  Phase 3 is structured so ACT runs ~100% busy and all matmul
    work hides underneath it.
  - tanh reads S^T tiles straight from PSUM (no DVE CAST staging); exp is
    batched per 2 j-blocks [128, 2048] and emits bf16 scores the AV
    matmul streams at 1 cycle/row.  The 2-block exp granularity matters:
    it never outruns the 2-deep PSUM S-tile ring, so the PE (stuck at the
    1.2GHz mid p-state because ACT paces it with sub-3us bursts) always
    has S work ready and ACT never stalls.
  - Lag-1 pipeline over all (i-half, head, jc-pair) units: the AV matmuls
    of the previous pair are emitted after S+tanh+exp of the current one.
  - All projection/attention operands are bf16 (weights pre-folded with
    norm_w on host, X^T via bf16 PE transposes), which also halves
    LDWEIGHTS time.  Norm-stat / scale-broadcast moving operands are f32r:
    fp32-moving matmuls run at 4 cyc/row instead of 1.
  - Attention runs i-half-major; phase 3.5/4 output-projection chunks for
    half 0 are sprinkled (every 6th pair) under half 1's ACT stream.
  - PSUM budget (8 banks): S^T tile ring 3x2 (shared with phase-3.5/4
    scratch halves) + AV accumulator 2.
  - Prologue: stats matmuls lag projections one slot (PE never waits on
    DVE squares); V projections run before the norm-apply so the rsqrt
    Ln/Exp chain hides; Wo's 2MB DMA is deferred past the startup path.
"""

import numpy as np
import ml_dtypes

import concourse.bass as bass
import concourse.mybir as mybir
import concourse.tile as tile
from concourse import bacc
from concourse.bass_utils import run_bass_kernel_spmd

P = 128
T = 2048          # tokens per batch
E = 1024          # embed dim
HL = 8            # heads per core (head-group)
D = 64            # head dim
CL = HL * D       # per-core qkv width (512)
NE = E // P       # 8 contraction chunks
NMC = CL // P     # 4 output chunks per projection (2 heads each)
NTB = T // P      # 16 token blocks
NSLOT = 2 * NMC   # 8 (proj, chunk) norm slots
EPS = float(np.finfo(np.float32).eps)

F32 = mybir.dt.float32
F32R = mybir.dt.float32r
BF16 = mybir.dt.bfloat16
AF = mybir.ActivationFunctionType
OP = mybir.AluOpType


def _r(ap):
    return ap.bitcast(F32R)


def _build_core_program():
    nc = bacc.Bacc(None, target_bir_lowering=False, debug=False)

    tokens_d = nc.dram_tensor("tokens_s", [T, E], F32, kind="ExternalInput")
    wq_d = nc.dram_tensor("wq_s", [E, CL], BF16, kind="ExternalInput")
    wk_d = nc.dram_tensor("wk_s", [E, CL], BF16, kind="ExternalInput")
    wv_d = nc.dram_tensor("wv_s", [E, CL], BF16, kind="ExternalInput")
    wo_d = nc.dram_tensor("wo_s", [CL, E], F32R, kind="ExternalInput")
    qg_d = nc.dram_tensor("qg_s", [P, NMC], F32, kind="ExternalInput")
    kg_d = nc.dram_tensor("kg_s", [P, NMC], F32, kind="ExternalInput")
    ident_d = nc.dram_tensor("ident_s", [P, P], BF16, kind="ExternalInput")
    oneslot_d = nc.dram_tensor("oneslot_s", [P, NSLOT, 2 * NSLOT], F32R,
                               kind="ExternalInput")
    sel16_d = nc.dram_tensor("sel16_s", [2 * NSLOT, NSLOT, P], F32R,
                             kind="ExternalInput")
    selh_d = nc.dram_tensor("selh_s", [HL, NMC * P], F32R, kind="ExternalInput")
    out_d = nc.dram_tensor("out_s", [T, E], F32, kind="ExternalOutput")

    with tile.TileContext(nc) as tc:
        _body(tc, tokens_d, wq_d, wk_d, wv_d, wo_d, qg_d, kg_d,
              ident_d, oneslot_d, sel16_d, selh_d, out_d)
    nc.compile()
    return nc


def _body(tc, tokens_d, wq_d, wk_d, wv_d, wo_d, qg_d, kg_d,
          ident_d, oneslot_d, sel16_d, selh_d, out_d):
    nc = tc.nc

    with tc.tile_pool(name="const", bufs=1) as const, \
         tc.tile_pool(name="big", bufs=1) as big:

        # Only ident is needed early (first transpose).  The other const
        # DMAs are emitted after phase-1's token loop so the first token
        # block's DMA descriptor isn't queued behind them at startup.
        ident = const.tile([P, P], BF16)
        nc.sync.dma_start(ident, ident_d.ap())
        oneslot = const.tile([P, NSLOT, 2 * NSLOT], F32R)
        sel16 = const.tile([2 * NSLOT, NSLOT, P], F32R)
        selh = const.tile([HL, NMC * P], F32R)
        qg = const.tile([P, NMC], F32)
        kg = const.tile([P, NMC], F32)

        # Persistent activations.
        qt = big.tile([P, NMC, T], BF16)          # Q^T (normed+scaled)
        kt = big.tile([P, NMC, T], BF16)          # K^T (normed+scaled)
        v65 = big.tile([P, NTB, HL * (D + 1)], BF16)  # V + ones col per head
        nc.gpsimd.memset(v65[:], 1.0)
        v65v = v65.rearrange("p a (h c) -> p a h c", c=D + 1)
        ot = big.tile([P, NMC, T], F32R)          # out^T per head (unnormed)
        lsb = big.tile([HL, T], F32)              # softmax denominators
        wosb = big.tile([P, NMC, E], F32R)
        rs_all = big.tile([P, NTB], F32)          # rmsnorm rsqrt per token blk

        # ---- Phase 1+2 in token halves (keeps X^T at half size) ----
        with tc.tile_pool(name="p12", bufs=3) as p12, \
             tc.tile_pool(name="xtp", bufs=2) as xtp, \
             tc.tile_pool(name="sqp", bufs=2) as sqp, \
             tc.tile_pool(name="wp", bufs=2) as wp, \
             tc.tile_pool(name="pp", bufs=3, space="PSUM") as pp, \
             tc.tile_pool(name="nsqp", bufs=1, space="PSUM") as nsqp, \
             tc.tile_pool(name="trp", bufs=2, space="PSUM") as trp:

            TH = T // 2          # 1024 tokens per half
            NTBH = TH // P       # 8 token blocks per half
            NITH = TH // 512     # 2 i-tiles per half

            for th in range(2):
                xt = xtp.tile([P, NE, TH], BF16, tag="xt")

                # Phase 1: X^T from RAW tokens.  The rmsnorm row-scale rs[i]
                # cancels exactly in the Q/K head-l2-norm, so only V needs it;
                # rs is computed here (chain fully hidden -- first consumed by
                # the phase-2b V copies much later) and applied per-partition
                # during the V PSUM->SBUF copy.  Transposes therefore wait
                # only on the plain bf16 cast of the token DMA.
                for tbl in range(NTBH):
                    tb = th * NTBH + tbl
                    tok = p12.tile([P, E], F32, tag="tok")
                    nc.sync.dma_start(tok, tokens_d.ap()[tb * P:(tb + 1) * P, :])
                    xs = p12.tile([P, E], BF16, tag="xs")
                    nc.vector.tensor_copy(xs, tok)
                    ssum = p12.tile([P, 1], F32, tag="ssum")
                    # in-place Square; only accum_out is used
                    nc.scalar.activation(tok, tok, AF.Square, accum_out=ssum)
                    mm_ = p12.tile([P, 1], F32, tag="mm_")
                    nc.vector.tensor_scalar(mm_, ssum, 1.0 / E, EPS,
                                            OP.mult, OP.add)
                    rcp = p12.tile([P, 1], F32, tag="rcp")
                    nc.vector.reciprocal(rcp, mm_)
                    nc.scalar.activation(rs_all[:, tb:tb + 1], rcp, AF.Sqrt)
                    for eg in range(NE // 4):
                        trps = trp.tile([P, 4, P], BF16, tag="trps")
                        for eo in range(4):
                            ec = eg * 4 + eo
                            nc.tensor.transpose(trps[:, eo, :],
                                                xs[:, ec * P:(ec + 1) * P], ident)
                        nc.vector.tensor_copy(
                            xt[:, eg * 4:(eg + 1) * 4, tbl * P:(tbl + 1) * P], trps)

                if th == 0:
                    # deferred const DMAs (first used by stats/apply, >30us in)
                    nc.sync.dma_start(oneslot, oneslot_d.ap())
                    nc.sync.dma_start(sel16, sel16_d.ap())
                    nc.sync.dma_start(selh, selh_d.ap())
                    nc.sync.dma_start(qg, qg_d.ap())
                    nc.sync.dma_start(kg, kg_d.ap())

                # Phase 2a: Q^T / K^T projections + batched norm stats.
                # Stat matmuls lag the projections one slot so the PE never
                # waits on the DVE-computed squares.
                nsq16 = nsqp.tile([2 * NSLOT, TH], F32, tag="nsq16")
                sq_done = []

                def emit_stats(slot, sqs):
                    for itl in range(NITH):
                        nc.tensor.matmul(
                            nsq16[:, itl * 512:(itl + 1) * 512],
                            oneslot[:, slot, :],
                            sqs[:, itl * 512:(itl + 1) * 512],
                            start=(slot == 0), stop=(slot == NSLOT - 1))

                for slot in range(NSLOT):
                    mc = slot % NMC
                    w_d = wq_d if slot < NMC else wk_d
                    dest = qt if slot < NMC else kt
                    wr = w_d.ap().rearrange("(ko p) m -> p ko m", p=P)
                    wblk = wp.tile([P, NE, P], BF16, tag="wblk")
                    nc.sync.dma_start(wblk, wr[:, :, mc * P:(mc + 1) * P])
                    sqs = sqp.tile([P, TH], F32R, tag="sqs")
                    for itl in range(NITH):
                        tsl = slice(th * TH + itl * 512, th * TH + (itl + 1) * 512)
                        prj = pp.tile([P, 512], F32, tag="pp")
                        for ec in range(NE):
                            nc.tensor.matmul(
                                prj, wblk[:, ec, :],
                                xt[:, ec, itl * 512:(itl + 1) * 512],
                                start=(ec == 0), stop=(ec == NE - 1))
                        nc.vector.tensor_copy(dest[:, mc, tsl], prj)
                        nc.vector.tensor_tensor(
                            sqs[:, itl * 512:(itl + 1) * 512],
                            dest[:, mc, tsl], dest[:, mc, tsl], OP.mult)
                    if sq_done:
                        emit_stats(*sq_done.pop())
                    sq_done.append((slot, sqs))
                emit_stats(*sq_done.pop())
                # one Ln+Exp pair for all slots of this half
                nsqs = p12.tile([2 * NSLOT, TH], F32, tag="nsqs")
                nc.vector.tensor_scalar_max(nsqs, nsq16, 1e-24)
                nc.scalar.activation(nsqs, nsqs, AF.Ln)
                rn16 = p12.tile([2 * NSLOT, TH], F32R, tag="rn16")
                nc.scalar.activation(rn16, nsqs, AF.Exp, scale=-0.5)

                # Phase 2b first: V projections hide the rsqrt chain latency
                wvsb = wp.tile([P, NE, CL], BF16, tag="wvsb", bufs=1)
                nc.sync.dma_start(wvsb, wv_d.ap().rearrange("(ko p) m -> p ko m", p=P))
                for tbl in range(NTBH):
                    tb = th * NTBH + tbl
                    pv = pp.tile([P, 512], F32, tag="pp")
                    for ec in range(NE):
                        nc.tensor.matmul(
                            pv, xt[:, ec, tbl * P:(tbl + 1) * P],
                            wvsb[:, ec, :],
                            start=(ec == 0), stop=(ec == NE - 1))
                    nc.vector.tensor_scalar_mul(
                        v65v[:, tb, :, 0:D],
                        pv.rearrange("p (h d) -> p h d", d=D),
                        rs_all[:, tb:tb + 1])

                # apply normalization * gamma-scale
                for slot, (g, dest) in enumerate(
                        ((qg, qt),) * NMC + ((kg, kt),) * NMC):
                    mc = slot % NMC
                    for itl in range(NITH):
                        tsl = slice(th * TH + itl * 512, th * TH + (itl + 1) * 512)
                        rnb = pp.tile([P, 512], F32, tag="pp")
                        nc.tensor.matmul(
                            rnb, sel16[:, slot, :],
                            rn16[:, itl * 512:(itl + 1) * 512],
                            start=True, stop=True)
                        nc.vector.scalar_tensor_tensor(
                            out=dest[:, mc, tsl], in0=dest[:, mc, tsl],
                            scalar=g[:, mc:mc + 1], in1=rnb,
                            op0=OP.mult, op1=OP.mult)

        # Wo staged late so its 2MB DMA stays off the startup critical path.
        nc.sync.dma_start(wosb, wo_d.ap().rearrange("(mc p) e -> p mc e", p=P))

        # ---- Phase 3: attention, i-half-major; ACT saturated ----
        with tc.tile_pool(name="ep", bufs=1) as ep, \
             tc.tile_pool(name="sap", bufs=2) as sap, \
             tc.tile_pool(name="drp", bufs=2) as drp, \
             tc.tile_pool(name="p4", bufs=2) as p4, \
             tc.tile_pool(name="spp", bufs=3, space="PSUM") as spp, \
             tc.tile_pool(name="avp", bufs=1, space="PSUM") as avp:

            av_t = [None]

            def emit_s(ip, h, p):
                mc, pr = h // 2, (h % 2) * 64
                sB = ep.tile([P, 2, 1024], F32, tag="sB", bufs=2)
                sA = sap.tile([P, 2, 1024], BF16, tag="sA", bufs=4)
                for jj in range(2):
                    jc = p * 2 + jj
                    sp = spp.tile([P, 1024], F32, tag="sp")
                    for hf in range(2):
                        nc.tensor.matmul(
                            sp[:, hf * 512:(hf + 1) * 512],
                            kt[pr:pr + 64, mc, jc * P:(jc + 1) * P],
                            qt[pr:pr + 64, mc,
                               ip * 1024 + hf * 512:ip * 1024 + (hf + 1) * 512],
                            start=True, stop=True)
                    nc.scalar.activation(sB[:, jj, :], sp, AF.Tanh,
                                         scale=1.0 / 50.0)
                nc.scalar.activation(
                    sA.rearrange("p a b -> p (a b)"),
                    sB.rearrange("p a b -> p (a b)"), AF.Exp, scale=6.25)
                return sA

            def emit_av(ip, h, p, sA):
                if p == 0:
                    av_t[0] = avp.tile([65, 1024], F32, tag="av", name="av")
                av = av_t[0]
                for jj in range(2):
                    jc = p * 2 + jj
                    for hf in range(2):
                        nc.tensor.matmul(
                            av[:, hf * 512:(hf + 1) * 512],
                            v65[:, jc, h * (D + 1):(h + 1) * (D + 1)],
                            sA[:, jj, hf * 512:(hf + 1) * 512],
                            start=(p == 0 and jj == 0),
                            stop=(p == 7 and jj == 1))

            def emit_drain(ip, h):
                mc, pr = h // 2, (h % 2) * 64
                ipsl = slice(ip * 1024, (ip + 1) * 1024)
                av = av_t[0]
                lrow = drp.tile([65, 1024], F32, tag="lrow")
                if pr == 0:
                    nc.vector.tensor_copy(ot[0:64, mc, ipsl], av[0:64, :])
                    nc.vector.tensor_copy(lrow[64:65, :], av[64:65, :])
                else:
                    nc.vector.tensor_copy(lrow[:], av[:])
                    nc.sync.dma_start(ot[64:128, mc, ipsl], _r(lrow[0:64, :]))
                nc.sync.dma_start(lsb[h:h + 1, ipsl], lrow[64:65, :])
            def emit_rli(ip):
                # approx_fast (~18 bits) is ample for softmax denominators;
                # the copy provides the f32r rounding the selh matmul requires
                rliF = p4.tile([HL, 1024], F32, tag="rliF")
                nc.vector.reciprocal_approx_fast(
                    rliF, lsb[:, ip * 1024:(ip + 1) * 1024])
                rli = p4.tile([HL, 1024], F32R, tag="rli")
                nc.vector.tensor_copy(rli, rliF)
                return rli

            def emit_p35(ip, il, rli):
                for mc in range(NMC):
                    csl = slice(ip * 1024 + il * 512, ip * 1024 + (il + 1) * 512)
                    rlb = spp.tile([P, 1024], F32, tag="sp", name="rlb")[:, 0:512]
                    nc.tensor.matmul(
                        rlb, selh[:, mc * P:(mc + 1) * P],
                        rli[:, il * 512:(il + 1) * 512],
                        start=True, stop=True)
                    nc.vector.tensor_tensor(
                        ot[:, mc, csl], ot[:, mc, csl], rlb, OP.mult)

            def emit_p4(ip, tbl):
                tb = ip * 8 + tbl
                res = p4.tile([P, E], F32, tag="res", bufs=3)
                for en in range(2):
                    ps = spp.tile([P, 1024], F32, tag="sp", name="ps")[:, 0:512]
                    for mc in range(NMC):
                        nc.tensor.matmul(
                            ps, ot[:, mc, tb * P:(tb + 1) * P],
                            wosb[:, mc, en * 512:(en + 1) * 512],
                            start=(mc == 0), stop=(mc == NMC - 1))
                    nc.vector.tensor_copy(res[:, en * 512:(en + 1) * 512], ps)
                nc.sync.dma_start(out_d.ap()[tb * P:(tb + 1) * P, :], res)

            def emit_out_chunk(ip, k, rli):
                # 10 chunks per i-half: 2x phase-3.5 + 8x phase-4 token blocks
                if k == 0:
                    emit_p35(ip, 0, rli)
                elif k == 5:
                    emit_p35(ip, 1, rli)
                elif k < 5:
                    emit_p4(ip, k - 1)
                else:
                    emit_p4(ip, k - 2)

            # Lag-1 software pipeline over all (ip, h, pair) units: AV of the
            # previous pair is emitted after S+tanh+exp of the current one, so
            # the PE always has ready matmul work while ACT runs and no ACT op
            # outruns the 2-deep PSUM S-tile ring.
            pairs = [(ip, h, p) for ip in range(2) for h in range(HL)
                     for p in range(8)]
            prev = None
            prev_sA = None
            rli0 = None
            chunk0 = 0
            slot = 0
            for pu in pairs:
                ip, h, p = pu
                sA = emit_s(ip, h, p)
                if prev is not None:
                    pip, ph, pg = prev
                    emit_av(pip, ph, pg, prev_sA)
                    if pg == 7:
                        emit_drain(pip, ph)
                        if pip == 0 and ph == HL - 1:
                            rli0 = emit_rli(0)
                    if rli0 is not None and chunk0 < 10:
                        slot += 1
                        # start at slot 8: keeps the first chunk clear of the
                        # drain+reciprocal burst at the ip boundary
                        if slot >= 8 and (slot - 8) % 6 == 0:
                            emit_out_chunk(0, chunk0, rli0)
                            chunk0 += 1
                prev, prev_sA = pu, sA
            emit_av(*prev, prev_sA)
            emit_drain(prev[0], prev[1])
            while chunk0 < 10:
                emit_out_chunk(0, chunk0, rli0)
                chunk0 += 1
            rli1 = emit_rli(1)
            # both 3.5 scale steps first: their rlb matmuls fill the PE while
            # the DVE ot-scaling the first phase-4 blocks wait on completes
            for k in (0, 5, 1, 2, 3, 4, 6, 7, 8, 9):
                emit_out_chunk(1, k, rli1)


_NC_CACHE = []


def get_program():
    if not _NC_CACHE:
        _NC_CACHE.append(_build_core_program())
    return _NC_CACHE[0]


def make_in_maps(tokens, norm_w, Wq, Wkv, Wo, q_gamma, k_gamma):
    tokens = np.asarray(tokens, np.float32)
    norm_w = np.asarray(norm_w, np.float32)
    Wq = np.asarray(Wq, np.float32)
    Wkv = np.asarray(Wkv, np.float32)
    Wo = np.asarray(Wo, np.float32)
    qg = ((np.asarray(q_gamma, np.float32) + 1.0) * np.float32(np.sqrt(D))).reshape(-1)
    kg = ((np.asarray(k_gamma, np.float32) + 1.0) * np.float32(np.sqrt(D))).reshape(-1)

    Wqf = norm_w[:, None] * Wq
    Wkf = norm_w[:, None] * Wkv[:, :E]
    Wvf = norm_w[:, None] * Wkv[:, E:]

    ident = np.eye(P, dtype=ml_dtypes.bfloat16)
    # oneslot[p, j, c]: ones-matmul lhsT for norm slot j -> rows 2j/2j+1
    oneslot = np.zeros((P, NSLOT, 2 * NSLOT), np.float32)
    sel16 = np.zeros((2 * NSLOT, NSLOT, P), np.float32)
    for j in range(NSLOT):
        oneslot[0:64, j, 2 * j] = 1.0
        oneslot[64:128, j, 2 * j + 1] = 1.0
        sel16[2 * j, j, 0:64] = 1.0
        sel16[2 * j + 1, j, 64:128] = 1.0
    selh = np.zeros((HL, NMC * P), np.float32)
    for h in range(HL):
        mc, pr = h // 2, (h % 2) * 64
        selh[h, mc * P + pr: mc * P + pr + 64] = 1.0

    in_maps = []
    for c in range(8):
        b, hg = c // 2, c % 2
        sl = slice(hg * CL, (hg + 1) * CL)
        in_maps.append({
            "tokens_s": np.ascontiguousarray(tokens[b]),
            "wq_s": np.ascontiguousarray(Wqf[:, sl]).astype(ml_dtypes.bfloat16),
            "wk_s": np.ascontiguousarray(Wkf[:, sl]).astype(ml_dtypes.bfloat16),
            "wv_s": np.ascontiguousarray(Wvf[:, sl]).astype(ml_dtypes.bfloat16),
            "wo_s": np.ascontiguousarray(Wo[sl, :]),
            "qg_s": np.ascontiguousarray(qg[sl].reshape(NMC, P).T),
            "kg_s": np.ascontiguousarray(kg[sl].reshape(NMC, P).T),
            "ident_s": ident,
            "oneslot_s": oneslot,
            "sel16_s": sel16,
            "selh_s": selh,
        })
    return in_maps


def gather_output(results):
    out = np.empty((4, T, E), np.float32)
    for b in range(4):
        out[b] = results[2 * b]["out_s"] + results[2 * b + 1]["out_s"]
    return out


def kernel(**inputs):
    nc = get_program()
    in_maps = make_in_maps(**inputs)
    res = run_bass_kernel_spmd(nc, in_maps, core_ids=list(range(8)))
    return gather_output(res.results)


# revision 73
# speedup vs baseline: 1.1890x; 1.1890x over previous
"""Trainium2 Bass kernel for softclamped multi-head attention.

Full computation (matches the jax reference):
  x = rmsnorm(tokens) * norm_w
  q = x @ Wq ; k, v = split(x @ Wkv)
  q, k head-l2norm * (gamma+1)*sqrt(d)
  sim = tanh((q k^T)/50)*50 / sqrt(d);  attn = softmax(sim);  out = (attn v) @ Wo

Sharding: 8 cores; core c -> batch c//2, head-group c%2 (8 of 16 heads).
Each core computes a partial output (its head-group's contribution to its
batch); host sums the two partials per batch.

Performance design (per core, engines at: PE 2.4GHz, ACT 1.2GHz, DVE 0.96GHz):
  - The ACT engine is the irreducible bottleneck: tanh+exp over the full
    8-head 2048x2048 score tensor = 2 passes x 262144 elem/partition
    ~ 500us.  Phase 3 is structured so ACT runs ~100% busy and all matmul
    work hides underneath it.
  - tanh reads S^T tiles straight from PSUM (no DVE CAST staging); exp is
    batched per 2 j-blocks [128, 2048] and emits bf16 scores the AV
    matmul streams at 1 cycle/row.  The 2-block exp granularity matters:
    it never outruns the 2-deep PSUM S-tile ring, so the PE (stuck at the
    1.2GHz mid p-state because ACT paces it with sub-3us bursts) always
    has S work ready and ACT never stalls.
  - Lag-1 pipeline over all (i-half, head, jc-pair) units: the AV matmuls
    of the previous pair are emitted after S+tanh+exp of the current one.
  - All projection/attention operands are bf16 (weights pre-folded with
    norm_w on host, X^T via bf16 PE transposes), which also halves
    LDWEIGHTS time.  Norm-stat / scale-broadcast moving operands are f32r:
    fp32-moving matmuls run at 4 cyc/row instead of 1.
  - Attention runs i-half-major; phase 3.5/4 output-projection chunks for
    half 0 are sprinkled (every 6th pair) under half 1's ACT stream.
  - PSUM budget (8 banks): S^T tile ring 3x2 (shared with phase-3.5/4
    scratch halves) + AV accumulator 2.
  - Prologue: stats matmuls lag projections one slot (PE never waits on
    DVE squares); V projections run before the norm-apply so the rsqrt
    Ln/Exp chain hides; Wo's 2MB DMA is deferred past the startup path.
"""

import numpy as np
import ml_dtypes

import concourse.bass as bass
import concourse.mybir as mybir
import concourse.tile as tile
from concourse import bacc
from concourse.bass_utils import run_bass_kernel_spmd

P = 128
T = 2048          # tokens per batch
E = 1024          # embed dim
HL = 8            # heads per core (head-group)
D = 64            # head dim
CL = HL * D       # per-core qkv width (512)
NE = E // P       # 8 contraction chunks
NMC = CL // P     # 4 output chunks per projection (2 heads each)
NTB = T // P      # 16 token blocks
NSLOT = 2 * NMC   # 8 (proj, chunk) norm slots
EPS = float(np.finfo(np.float32).eps)

F32 = mybir.dt.float32
F32R = mybir.dt.float32r
BF16 = mybir.dt.bfloat16
AF = mybir.ActivationFunctionType
OP = mybir.AluOpType


def _r(ap):
    return ap.bitcast(F32R)


def _build_core_program():
    nc = bacc.Bacc(None, target_bir_lowering=False, debug=False)

    tokens_d = nc.dram_tensor("tokens_s", [T, E], F32, kind="ExternalInput")
    wq_d = nc.dram_tensor("wq_s", [E, CL], BF16, kind="ExternalInput")
    wk_d = nc.dram_tensor("wk_s", [E, CL], BF16, kind="ExternalInput")
    wv_d = nc.dram_tensor("wv_s", [E, CL], BF16, kind="ExternalInput")
    wo_d = nc.dram_tensor("wo_s", [CL, E], F32R, kind="ExternalInput")
    qg_d = nc.dram_tensor("qg_s", [P, NMC], F32, kind="ExternalInput")
    kg_d = nc.dram_tensor("kg_s", [P, NMC], F32, kind="ExternalInput")
    ident_d = nc.dram_tensor("ident_s", [P, P], BF16, kind="ExternalInput")
    oneslot_d = nc.dram_tensor("oneslot_s", [P, NSLOT, 2 * NSLOT], F32R,
                               kind="ExternalInput")
    sel16_d = nc.dram_tensor("sel16_s", [2 * NSLOT, NSLOT, P], F32R,
                             kind="ExternalInput")
    selh_d = nc.dram_tensor("selh_s", [HL, NMC * P], F32R, kind="ExternalInput")
    out_d = nc.dram_tensor("out_s", [T, E], F32, kind="ExternalOutput")

    with tile.TileContext(nc) as tc:
        _body(tc, tokens_d, wq_d, wk_d, wv_d, wo_d, qg_d, kg_d,
              ident_d, oneslot_d, sel16_d, selh_d, out_d)
    nc.compile()
    return nc


def _body(tc, tokens_d, wq_d, wk_d, wv_d, wo_d, qg_d, kg_d,
          ident_d, oneslot_d, sel16_d, selh_d, out_d):
    nc = tc.nc

    with tc.tile_pool(name="const", bufs=1) as const, \
         tc.tile_pool(name="big", bufs=1) as big:

        # Only ident is needed early (first transpose).  The other const
        # DMAs are emitted after phase-1's token loop so the first token
        # block's DMA descriptor isn't queued behind them at startup.
        ident = const.tile([P, P], BF16)
        nc.sync.dma_start(ident, ident_d.ap())
        oneslot = const.tile([P, NSLOT, 2 * NSLOT], F32R)
        sel16 = const.tile([2 * NSLOT, NSLOT, P], F32R)
        selh = const.tile([HL, NMC * P], F32R)
        qg = const.tile([P, NMC], F32)
        kg = const.tile([P, NMC], F32)

        # Persistent activations.
        qt = big.tile([P, NMC, T], BF16)          # Q^T (normed+scaled)
        kt = big.tile([P, NMC, T], BF16)          # K^T (normed+scaled)
        v65 = big.tile([P, NTB, HL * (D + 1)], BF16)  # V + ones col per head
        nc.gpsimd.memset(v65[:], 1.0)
        v65v = v65.rearrange("p a (h c) -> p a h c", c=D + 1)
        ot = big.tile([P, NMC, T], F32R)          # out^T per head (unnormed)
        lsb = big.tile([HL, T], F32)              # softmax denominators
        wosb = big.tile([P, NMC, E], F32R)
        rs_all = big.tile([P, NTB], F32)          # rmsnorm rsqrt per token blk

        # ---- Phase 1+2 in token halves (keeps X^T at half size) ----
        with tc.tile_pool(name="p12", bufs=3) as p12, \
             tc.tile_pool(name="xtp", bufs=2) as xtp, \
             tc.tile_pool(name="sqp", bufs=2) as sqp, \
             tc.tile_pool(name="wp", bufs=2) as wp, \
             tc.tile_pool(name="pp", bufs=3, space="PSUM") as pp, \
             tc.tile_pool(name="nsqp", bufs=1, space="PSUM") as nsqp, \
             tc.tile_pool(name="trp", bufs=2, space="PSUM") as trp:

            TH = T // 2          # 1024 tokens per half
            NTBH = TH // P       # 8 token blocks per half
            NITH = TH // 512     # 2 i-tiles per half

            for th in range(2):
                xt = xtp.tile([P, NE, TH], BF16, tag="xt")

                # Phase 1: X^T from RAW tokens.  The rmsnorm row-scale rs[i]
                # cancels exactly in the Q/K head-l2-norm, so only V needs it;
                # rs is computed here (chain fully hidden -- first consumed by
                # the phase-2b V copies much later) and applied per-partition
                # during the V PSUM->SBUF copy.  Transposes therefore wait
                # only on the plain bf16 cast of the token DMA.
                for tbl in range(NTBH):
                    tb = th * NTBH + tbl
                    tok = p12.tile([P, E], F32, tag="tok")
                    nc.sync.dma_start(tok, tokens_d.ap()[tb * P:(tb + 1) * P, :])
                    xs = p12.tile([P, E], BF16, tag="xs")
                    # cast on ACT (idle here) -- phase-1 is DVE-paced
                    nc.scalar.copy(xs, tok)
                    ssum = p12.tile([P, 1], F32, tag="ssum")
                    # in-place Square; only accum_out is used
                    nc.scalar.activation(tok, tok, AF.Square, accum_out=ssum)
                    mm_ = p12.tile([P, 1], F32, tag="mm_")
                    nc.vector.tensor_scalar(mm_, ssum, 1.0 / E, EPS,
                                            OP.mult, OP.add)
                    rcp = p12.tile([P, 1], F32, tag="rcp")
                    nc.vector.reciprocal(rcp, mm_)
                    nc.scalar.activation(rs_all[:, tb:tb + 1], rcp, AF.Sqrt)
                    for eg in range(NE // 4):
                        trps = trp.tile([P, 4, P], BF16, tag="trps")
                        for eo in range(4):
                            ec = eg * 4 + eo
                            nc.tensor.transpose(trps[:, eo, :],
                                                xs[:, ec * P:(ec + 1) * P], ident)
                        nc.vector.tensor_copy(
                            xt[:, eg * 4:(eg + 1) * 4, tbl * P:(tbl + 1) * P], trps)

                if th == 0:
                    # deferred const DMAs (first used by stats/apply, >30us in)
                    nc.sync.dma_start(oneslot, oneslot_d.ap())
                    nc.sync.dma_start(sel16, sel16_d.ap())
                    nc.sync.dma_start(selh, selh_d.ap())
                    nc.sync.dma_start(qg, qg_d.ap())
                    nc.sync.dma_start(kg, kg_d.ap())

                # Phase 2a: Q^T / K^T projections + batched norm stats.
                # Stat matmuls lag the projections one slot so the PE never
                # waits on the DVE-computed squares.
                nsq16 = nsqp.tile([2 * NSLOT, TH], F32, tag="nsq16")
                sq_done = []

                def emit_stats(slot, sqs):
                    for itl in range(NITH):
                        nc.tensor.matmul(
                            nsq16[:, itl * 512:(itl + 1) * 512],
                            oneslot[:, slot, :],
                            sqs[:, itl * 512:(itl + 1) * 512],
                            start=(slot == 0), stop=(slot == NSLOT - 1))

                for slot in range(NSLOT):
                    mc = slot % NMC
                    w_d = wq_d if slot < NMC else wk_d
                    dest = qt if slot < NMC else kt
                    wr = w_d.ap().rearrange("(ko p) m -> p ko m", p=P)
                    wblk = wp.tile([P, NE, P], BF16, tag="wblk")
                    nc.sync.dma_start(wblk, wr[:, :, mc * P:(mc + 1) * P])
                    sqs = sqp.tile([P, TH], F32R, tag="sqs")
                    for itl in range(NITH):
                        tsl = slice(th * TH + itl * 512, th * TH + (itl + 1) * 512)
                        prj = pp.tile([P, 512], F32, tag="pp")
                        for ec in range(NE):
                            nc.tensor.matmul(
                                prj, wblk[:, ec, :],
                                xt[:, ec, itl * 512:(itl + 1) * 512],
                                start=(ec == 0), stop=(ec == NE - 1))
                        nc.vector.tensor_copy(dest[:, mc, tsl], prj)
                        nc.vector.tensor_tensor(
                            sqs[:, itl * 512:(itl + 1) * 512],
                            dest[:, mc, tsl], dest[:, mc, tsl], OP.mult)
                    if sq_done:
                        emit_stats(*sq_done.pop())
                    sq_done.append((slot, sqs))
                emit_stats(*sq_done.pop())
                # one Ln+Exp pair for all slots of this half
                nsqs = p12.tile([2 * NSLOT, TH], F32, tag="nsqs")
                nc.vector.tensor_scalar_max(nsqs, nsq16, 1e-24)
                nc.scalar.activation(nsqs, nsqs, AF.Ln)
                rn16 = p12.tile([2 * NSLOT, TH], F32R, tag="rn16")
                nc.scalar.activation(rn16, nsqs, AF.Exp, scale=-0.5)

                # Phase 2b first: V projections hide the rsqrt chain latency
                wvsb = wp.tile([P, NE, CL], BF16, tag="wvsb", bufs=1)
                nc.sync.dma_start(wvsb, wv_d.ap().rearrange("(ko p) m -> p ko m", p=P))
                for tbl in range(NTBH):
                    tb = th * NTBH + tbl
                    pv = pp.tile([P, 512], F32, tag="pp")
                    for ec in range(NE):
                        nc.tensor.matmul(
                            pv, xt[:, ec, tbl * P:(tbl + 1) * P],
                            wvsb[:, ec, :],
                            start=(ec == 0), stop=(ec == NE - 1))
                    nc.vector.tensor_scalar_mul(
                        v65v[:, tb, :, 0:D],
                        pv.rearrange("p (h d) -> p h d", d=D),
                        rs_all[:, tb:tb + 1])

                # apply normalization * gamma-scale
                for slot, (g, dest) in enumerate(
                        ((qg, qt),) * NMC + ((kg, kt),) * NMC):
                    mc = slot % NMC
                    for itl in range(NITH):
                        tsl = slice(th * TH + itl * 512, th * TH + (itl + 1) * 512)
                        rnb = pp.tile([P, 512], F32, tag="pp")
                        nc.tensor.matmul(
                            rnb, sel16[:, slot, :],
                            rn16[:, itl * 512:(itl + 1) * 512],
                            start=True, stop=True)
                        nc.vector.scalar_tensor_tensor(
                            out=dest[:, mc, tsl], in0=dest[:, mc, tsl],
                            scalar=g[:, mc:mc + 1], in1=rnb,
                            op0=OP.mult, op1=OP.mult)

        # Wo staged late so its 2MB DMA stays off the startup critical path.
        nc.sync.dma_start(wosb, wo_d.ap().rearrange("(mc p) e -> p mc e", p=P))

        # ---- Phase 3: attention, i-half-major; ACT saturated ----
        with tc.tile_pool(name="ep", bufs=1) as ep, \
             tc.tile_pool(name="sap", bufs=2) as sap, \
             tc.tile_pool(name="drp", bufs=2) as drp, \
             tc.tile_pool(name="p4", bufs=2) as p4, \
             tc.tile_pool(name="spp", bufs=3, space="PSUM") as spp, \
             tc.tile_pool(name="avp", bufs=1, space="PSUM") as avp:

            av_t = [None]

            def emit_s(ip, h, p):
                mc, pr = h // 2, (h % 2) * 64
                sB = ep.tile([P, 2, 1024], F32, tag="sB", bufs=2)
                sA = sap.tile([P, 2, 1024], BF16, tag="sA", bufs=4)
                for jj in range(2):
                    jc = p * 2 + jj
                    sp = spp.tile([P, 1024], F32, tag="sp")
                    for hf in range(2):
                        nc.tensor.matmul(
                            sp[:, hf * 512:(hf + 1) * 512],
                            kt[pr:pr + 64, mc, jc * P:(jc + 1) * P],
                            qt[pr:pr + 64, mc,
                               ip * 1024 + hf * 512:ip * 1024 + (hf + 1) * 512],
                            start=True, stop=True)
                    nc.scalar.activation(sB[:, jj, :], sp, AF.Tanh,
                                         scale=1.0 / 50.0)
                nc.scalar.activation(
                    sA.rearrange("p a b -> p (a b)"),
                    sB.rearrange("p a b -> p (a b)"), AF.Exp, scale=6.25)
                return sA

            def emit_av(ip, h, p, sA):
                if p == 0:
                    av_t[0] = avp.tile([65, 1024], F32, tag="av", name="av")
                av = av_t[0]
                for jj in range(2):
                    jc = p * 2 + jj
                    for hf in range(2):
                        nc.tensor.matmul(
                            av[:, hf * 512:(hf + 1) * 512],
                            v65[:, jc, h * (D + 1):(h + 1) * (D + 1)],
                            sA[:, jj, hf * 512:(hf + 1) * 512],
                            start=(p == 0 and jj == 0),
                            stop=(p == 7 and jj == 1))

            def emit_drain(ip, h):
                mc, pr = h // 2, (h % 2) * 64
                ipsl = slice(ip * 1024, (ip + 1) * 1024)
                av = av_t[0]
                lrow = drp.tile([65, 1024], F32, tag="lrow")
                if pr == 0:
                    nc.vector.tensor_copy(ot[0:64, mc, ipsl], av[0:64, :])
                    nc.vector.tensor_copy(lrow[64:65, :], av[64:65, :])
                else:
                    nc.vector.tensor_copy(lrow[:], av[:])
                    nc.sync.dma_start(ot[64:128, mc, ipsl], _r(lrow[0:64, :]))
                nc.sync.dma_start(lsb[h:h + 1, ipsl], lrow[64:65, :])
            def emit_rli(ip):
                # approx_fast (~18 bits) is ample for softmax denominators;
                # the copy provides the f32r rounding the selh matmul requires
                rliF = p4.tile([HL, 1024], F32, tag="rliF")
                nc.vector.reciprocal_approx_fast(
                    rliF, lsb[:, ip * 1024:(ip + 1) * 1024])
                rli = p4.tile([HL, 1024], F32R, tag="rli")
                nc.vector.tensor_copy(rli, rliF)
                return rli

            def emit_p35(ip, il, rli):
                for mc in range(NMC):
                    csl = slice(ip * 1024 + il * 512, ip * 1024 + (il + 1) * 512)
                    rlb = spp.tile([P, 1024], F32, tag="sp", name="rlb")[:, 0:512]
                    nc.tensor.matmul(
                        rlb, selh[:, mc * P:(mc + 1) * P],
                        rli[:, il * 512:(il + 1) * 512],
                        start=True, stop=True)
                    nc.vector.tensor_tensor(
                        ot[:, mc, csl], ot[:, mc, csl], rlb, OP.mult)

            def emit_p4(ip, tbl):
                tb = ip * 8 + tbl
                res = p4.tile([P, E], F32, tag="res", bufs=3)
                for en in range(2):
                    ps = spp.tile([P, 1024], F32, tag="sp", name="ps")[:, 0:512]
                    for mc in range(NMC):
                        nc.tensor.matmul(
                            ps, ot[:, mc, tb * P:(tb + 1) * P],
                            wosb[:, mc, en * 512:(en + 1) * 512],
                            start=(mc == 0), stop=(mc == NMC - 1))
                    nc.vector.tensor_copy(res[:, en * 512:(en + 1) * 512], ps)
                nc.sync.dma_start(out_d.ap()[tb * P:(tb + 1) * P, :], res)

            def emit_out_chunk(ip, k, rli):
                # 10 chunks per i-half: 2x phase-3.5 + 8x phase-4 token blocks
                if k == 0:
                    emit_p35(ip, 0, rli)
                elif k == 5:
                    emit_p35(ip, 1, rli)
                elif k < 5:
                    emit_p4(ip, k - 1)
                else:
                    emit_p4(ip, k - 2)

            # Lag-1 software pipeline over all (ip, h, pair) units: AV of the
            # previous pair is emitted after S+tanh+exp of the current one, so
            # the PE always has ready matmul work while ACT runs and no ACT op
            # outruns the 2-deep PSUM S-tile ring.
            pairs = [(ip, h, p) for ip in range(2) for h in range(HL)
                     for p in range(8)]
            prev = None
            prev_sA = None
            rli0 = None
            chunk0 = 0
            slot = 0
            for pu in pairs:
                ip, h, p = pu
                sA = emit_s(ip, h, p)
                if prev is not None:
                    pip, ph, pg = prev
                    emit_av(pip, ph, pg, prev_sA)
                    if pg == 7:
                        emit_drain(pip, ph)
                        if pip == 0 and ph == HL - 1:
                            rli0 = emit_rli(0)
                    if rli0 is not None and chunk0 < 10:
                        slot += 1
                        # start at slot 8: keeps the first chunk clear of the
                        # drain+reciprocal burst at the ip boundary
                        if slot >= 8 and (slot - 8) % 6 == 0:
                            emit_out_chunk(0, chunk0, rli0)
                            chunk0 += 1
                prev, prev_sA = pu, sA
            emit_av(*prev, prev_sA)
            emit_drain(prev[0], prev[1])
            while chunk0 < 10:
                emit_out_chunk(0, chunk0, rli0)
                chunk0 += 1
            rli1 = emit_rli(1)
            # both 3.5 scale steps first: their rlb matmuls fill the PE while
            # the DVE ot-scaling the first phase-4 blocks wait on completes
            for k in (0, 5, 1, 2, 3, 4, 6, 7, 8, 9):
                emit_out_chunk(1, k, rli1)


_NC_CACHE = []


def get_program():
    if not _NC_CACHE:
        _NC_CACHE.append(_build_core_program())
    return _NC_CACHE[0]


def make_in_maps(tokens, norm_w, Wq, Wkv, Wo, q_gamma, k_gamma):
    tokens = np.asarray(tokens, np.float32)
    norm_w = np.asarray(norm_w, np.float32)
    Wq = np.asarray(Wq, np.float32)
    Wkv = np.asarray(Wkv, np.float32)
    Wo = np.asarray(Wo, np.float32)
    qg = ((np.asarray(q_gamma, np.float32) + 1.0) * np.float32(np.sqrt(D))).reshape(-1)
    kg = ((np.asarray(k_gamma, np.float32) + 1.0) * np.float32(np.sqrt(D))).reshape(-1)

    Wqf = norm_w[:, None] * Wq
    Wkf = norm_w[:, None] * Wkv[:, :E]
    Wvf = norm_w[:, None] * Wkv[:, E:]

    ident = np.eye(P, dtype=ml_dtypes.bfloat16)
    # oneslot[p, j, c]: ones-matmul lhsT for norm slot j -> rows 2j/2j+1
    oneslot = np.zeros((P, NSLOT, 2 * NSLOT), np.float32)
    sel16 = np.zeros((2 * NSLOT, NSLOT, P), np.float32)
    for j in range(NSLOT):
        oneslot[0:64, j, 2 * j] = 1.0
        oneslot[64:128, j, 2 * j + 1] = 1.0
        sel16[2 * j, j, 0:64] = 1.0
        sel16[2 * j + 1, j, 64:128] = 1.0
    selh = np.zeros((HL, NMC * P), np.float32)
    for h in range(HL):
        mc, pr = h // 2, (h % 2) * 64
        selh[h, mc * P + pr: mc * P + pr + 64] = 1.0

    in_maps = []
    for c in range(8):
        b, hg = c // 2, c % 2
        sl = slice(hg * CL, (hg + 1) * CL)
        in_maps.append({
            "tokens_s": np.ascontiguousarray(tokens[b]),
            "wq_s": np.ascontiguousarray(Wqf[:, sl]).astype(ml_dtypes.bfloat16),
            "wk_s": np.ascontiguousarray(Wkf[:, sl]).astype(ml_dtypes.bfloat16),
            "wv_s": np.ascontiguousarray(Wvf[:, sl]).astype(ml_dtypes.bfloat16),
            "wo_s": np.ascontiguousarray(Wo[sl, :]),
            "qg_s": np.ascontiguousarray(qg[sl].reshape(NMC, P).T),
            "kg_s": np.ascontiguousarray(kg[sl].reshape(NMC, P).T),
            "ident_s": ident,
            "oneslot_s": oneslot,
            "sel16_s": sel16,
            "selh_s": selh,
        })
    return in_maps


def gather_output(results):
    out = np.empty((4, T, E), np.float32)
    for b in range(4):
        out[b] = results[2 * b]["out_s"] + results[2 * b + 1]["out_s"]
    return out


def kernel(**inputs):
    nc = get_program()
    in_maps = make_in_maps(**inputs)
    res = run_bass_kernel_spmd(nc, in_maps, core_ids=list(range(8)))
    return gather_output(res.results)
